# revision 1
# baseline (speedup 1.0000x reference)
"""LeNet-C3-style masked 5x5 VALID conv on Trainium2, batch-sharded over 8 cores.

x [32,6,512,512] f32, weight [16,6,5,5] (masked by the C3 connectivity
table), bias [16] -> out [32,16,508,508] f32.

Per-core scheme (4 images each):
  - Window = 8 output rows (y0..y0+7); needs input rows y0..y0+11.
  - SBUF x slot [72 partitions = (ch<6, row<12) c-major, 512] loaded with one
    ~147KB DMA per (window, img).
  - Per (window, img) "group" k: 5 matmuls (one per kernel-column dx, rhs
    free-dim offset dx) accumulate into a PSUM bank [128 = (oc,yhat), 508].
    Stationary lhsT [72, 128] per dx is host-precomputed: masked weight
    scattered into the (ch,row) x (oc,yhat) banded layout.
  - ScalarE Identity activation evicts PSUM -> SBUF adding per-partition bias.
  - One DMA store per group.

Raw bass (no Tile): three engine streams (SP: DMA issue, PE: matmuls,
ACT: evictions) with standalone wait_ge instructions and cumulative
semaphore thresholds. Slot counts: 12 x slots (3 windows), 8 PSUM banks,
6 output slots.
"""

import numpy as np

# LeNet-5 C3 connectivity: input maps feeding each of the 16 output maps.
MAP_S2 = [[0, 1, 2], [1, 2, 3], [2, 3, 4], [3, 4, 5], [0, 4, 5], [0, 1, 5],
          [0, 1, 2, 3], [1, 2, 3, 4], [2, 3, 4, 5], [0, 3, 4, 5], [0, 1, 4, 5],
          [0, 1, 2, 5], [0, 1, 3, 4], [1, 2, 4, 5], [0, 2, 3, 5],
          [0, 1, 2, 3, 4, 5]]

B, C, H, W = 32, 6, 512, 512
OC, KH, KW = 16, 5, 5
OH, OW = H - KH + 1, W - KW + 1  # 508, 508
NCORES = 8
BPC = B // NCORES  # 4 images per core
YB = 8             # output rows per window
RR = YB + KH - 1   # 12 input rows per window
CP = 10            # channels padded 6->10 so loads hit 120 partitions
NP = RR * CP       # 120 rhs partitions (partition = r*10 + c, c<6 live)
M = OC * YB        # 128 psum partitions

# window start rows: 0,8,...,496 then a tail window at 500 (re-computes
# rows 500..503 with identical values; input rows 500..511 stay in bounds)
YS = list(range(0, OH - YB, YB)) + [OH - YB]
NW = len(YS)          # 64 windows
NG = NW * BPC         # 256 groups (window, img)
PSB = 8               # psum banks in flight
XW = 4                # x slot windows (16 slots)
LA = 3                # load lookahead (windows)
OSL = 6               # output slots

_NC_CACHE = {}
# matmul dtype config: dt = moving (rhs) dtype, w_dt = stationary dtype
# (None -> same as dt). float32r streams at full rate for N>=256.
CFG = {"dt": "float32r", "w_dt": None, "warm": False}


def _np_dt(name):
    if name in (None, "float32", "float32r"):
        return np.float32
    import ml_dtypes
    return np.dtype(getattr(ml_dtypes, name))


def _conn_mask():
    m = np.zeros((OC, C), dtype=np.float32)
    for i, conn in enumerate(MAP_S2):
        m[i, conn] = 1.0
    return m


def build_nc(dt_name="float32r", w_dt_name=None, reps=1, warm=False):
    import concourse.bass as bass
    import concourse.mybir as mybir
    from contextlib import ExitStack

    MMDT = getattr(mybir.dt, dt_name)
    WDT = getattr(mybir.dt, w_dt_name or dt_name)
    F32 = mybir.dt.float32
    BF16 = mybir.dt.bfloat16
    IDENT = mybir.ActivationFunctionType.Identity
    TW = reps * NW      # total windows across reps (timing amplification)
    TG = TW * BPC       # total groups
    NSL = 4             # paired output slots (each holds 2 groups)

    nc = bass.Bass()
    x_t = nc.dram_tensor("x", [BPC, C, H, W], MMDT, kind="ExternalInput")
    w_t = nc.dram_tensor("wstat", [NP, KW * M], WDT, kind="ExternalInput")
    b_t = nc.dram_tensor("biasrep", [M, 1], F32, kind="ExternalInput")
    z_t = nc.dram_tensor("zpad", [RR, XW * BPC * W], MMDT,
                         kind="ExternalInput")
    out_t = nc.dram_tensor("out", [BPC, OC, OH, OW], F32, kind="ExternalOutput")

    with ExitStack() as ctx:
        wt = ctx.enter_context(nc.sbuf_tensor("wt", [NP, KW * M], WDT))
        bt = ctx.enter_context(nc.sbuf_tensor("bt", [M, 1], F32))
        xt = ctx.enter_context(nc.sbuf_tensor("xt", [NP, XW * BPC, W], MMDT))
        ot = ctx.enter_context(nc.sbuf_tensor("ot", [M, NSL, 2, OW], F32))
        wb = ctx.enter_context(nc.sbuf_tensor("wb", [1, 2], BF16))
        # one PSUM tensor, bank-aligned 512-wide slices; cols 508..511 of
        # bank 0 double as the bf16 HAM-warmer target.
        pst = ctx.enter_context(nc.psum_tensor("pst", [M, PSB, 512], F32))
        wt_sem = ctx.enter_context(nc.semaphore("wt_sem"))
        bt_sem = ctx.enter_context(nc.semaphore("bt_sem"))
        # per-slot-group lane sems: same-lane DMA completions are ordered
        # through the slot-recycle chain, so thresholds are race-free.
        x_sems = [ctx.enter_context(nc.semaphore(f"x_sem{i}"))
                  for i in range(XW)]
        st_sems = [ctx.enter_context(nc.semaphore(f"st_sem{i}"))
                   for i in range(NSL)]
        ps_sem = ctx.enter_context(nc.semaphore("ps_sem"))
        ev_sem = ctx.enter_context(nc.semaphore("ev_sem"))
        z_sem = ctx.enter_context(nc.semaphore("z_sem"))
        block = ctx.enter_context(nc.Block())

        XFREE = XW * BPC * W  # xt free elements per partition

        def load_window(sync, wp):
            y0 = YS[wp % NW]
            for img in range(BPC):
                slot = (wp % XW) * BPC + img
                for c in range(C):
                    # dest partitions c, c+10, ..., c+110 (one per row)
                    dst = bass.AP(xt, c * XFREE + slot * W,
                                  [[XFREE * CP, RR], [1, W]])
                    sync.dma_start(
                        out=dst, in_=x_t[img, c, y0:y0 + RR, :],
                    ).then_inc(x_sems[wp % XW], 16)

        @block.sync
        def _(sync):
            sync.dma_start(out=wt[:, :], in_=w_t[:, :]).then_inc(wt_sem, 16)
            sync.dma_start(out=bt[:, :], in_=b_t[:, :]).then_inc(bt_sem, 16)
            # zero the pad partitions (c=6..9) once so they multiply as 0
            for c in range(C, CP):
                dst = bass.AP(xt, c * XFREE, [[XFREE * CP, RR], [1, XFREE]])
                sync.dma_start(out=dst, in_=z_t[:, :]).then_inc(wt_sem, 16)
            for wp in range(min(LA, TW)):
                load_window(sync, wp)
            for w in range(TW):
                wp = w + LA
                if wp < TW:
                    # slots (wp%XW) last read by window wp-XW's matmul groups
                    if wp >= XW:
                        sync.wait_ge(ps_sem, BPC * (wp - XW) + BPC)
                    load_window(sync, wp)
                for img in range(BPC):
                    k = BPC * w + img
                    sync.wait_ge(ev_sem, k // 2 + 1)
                    y0 = YS[w % NW]
                    dst = bass.AP(
                        out_t,
                        img * OC * OH * OW + y0 * OW,
                        [[OH * OW, OC], [OW, YB], [1, OW]],
                    )
                    sync.dma_start(out=dst, in_=ot[:, (k // 2) % NSL, k % 2, :]
                                   ).then_inc(st_sems[(k // 2) % NSL], 16)

        @block.tensor
        def _(tensor):
            assert TW % 2 == 0 and 2 * BPC == PSB
            tensor.wait_ge(wt_sem, 16 * (1 + CP - C))
            # window pairs: 8 groups fill all 8 psum banks, dx outer over
            # all 8 so each stationary is loaded once per 8 matmuls.
            for p in range(TW // 2):
                w0 = 2 * p
                for w in (w0, w0 + 1):
                    tensor.wait_ge(x_sems[w % XW],
                                   16 * BPC * C * (w // XW + 1))
                k0 = BPC * w0
                for dx in range(KW):
                    for j in range(2 * BPC):
                        k = k0 + j
                        if dx == 0 and k >= PSB:
                            tensor.wait_ge(ev_sem, (k - PSB) // 2 + 1)
                        w = w0 + j // BPC
                        img = j % BPC
                        slot = (w % XW) * BPC + img
                        mm = tensor.matmul(
                            pst[:, k % PSB, 0:OW],
                            wt[:, dx * M:(dx + 1) * M],
                            xt[:, slot, dx:dx + OW],
                            start=(dx == 0),
                            stop=(dx == KW - 1),
                        )
                        if dx == KW - 1:
                            mm.then_inc(ps_sem, 1)
                    if warm:
                        # tiny bf16 matmul: keeps the PE HAM activity
                        # monitor busy so fp32r streams at the warm clock
                        tensor.matmul(
                            pst[0:1, 0, 508:509],
                            wb[0:1, 0:1], wb[0:1, 1:2],
                            start=True, stop=True,
                            skip_group_check=True,
                        )

        @block.scalar
        def _(scalar):
            scalar.wait_ge(bt_sem, 16)
            for kp in range(TG // 2):
                scalar.wait_ge(ps_sem, 2 * kp + 2)
                if kp >= NSL:
                    scalar.wait_ge(st_sems[kp % NSL], 32 * (kp // NSL))
                b0 = (2 * kp) % PSB
                scalar.activation(
                    ot[:, kp % NSL, :, :], pst[:, b0:b0 + 2, 0:OW], IDENT,
                    bias=bt[:, :], scale=1.0,
                ).then_inc(ev_sem, 1)

    return nc


def _prep_wstat(weight):
    """[120, 5*128]: wstat[(rh*10+c), dx*128 + (oc*8+yy)] = wm[oc,c,rh-yy,dx]
    (c >= 6 rows stay zero — padding for DMA port utilization)."""
    wm = (weight.astype(np.float32) * _conn_mask()[:, :, None, None])
    wt = wm.transpose(3, 1, 0, 2)  # [dx, c, oc, dy]
    ws = np.zeros((KW, RR, CP, OC, YB), dtype=np.float32)
    for dy in range(KH):
        for yy in range(YB):
            ws[:, yy + dy, :C, :, yy] = wt[:, :, :, dy].transpose(0, 1, 2)
    out = np.ascontiguousarray(
        ws.reshape(KW, NP, M).transpose(1, 0, 2).reshape(NP, KW * M))
    return out.astype(_np_dt(CFG["w_dt"] or CFG["dt"]))


def kernel(x, weight, bias):
    from concourse.bass_utils import run_bass_kernel_spmd

    x = np.ascontiguousarray(np.asarray(x, dtype=np.float32)
                             .astype(_np_dt(CFG["dt"])))
    wstat = _prep_wstat(weight)
    biasrep = np.ascontiguousarray(
        np.repeat(bias.astype(np.float32), YB).reshape(M, 1))

    key = (CFG["dt"], CFG["w_dt"], CFG["warm"])
    if _NC_CACHE.get("key") != key:
        _NC_CACHE["nc"] = build_nc(dt_name=CFG["dt"], w_dt_name=CFG["w_dt"],
                                   warm=CFG["warm"])
        _NC_CACHE["key"] = key
    nc = _NC_CACHE["nc"]

    zpad = np.zeros((RR, XW * BPC * W), dtype=_np_dt(CFG["dt"]))
    in_maps = [
        {"x": x[c * BPC:(c + 1) * BPC], "wstat": wstat, "biasrep": biasrep,
         "zpad": zpad}
        for c in range(NCORES)
    ]
    res = run_bass_kernel_spmd(nc, in_maps, list(range(NCORES)))
    return np.concatenate([res.results[c]["out"] for c in range(NCORES)],
                          axis=0)



# revision 29
# speedup vs baseline: 7.2995x; 7.2995x over previous
"""LeNet-C3-style masked 5x5 VALID conv on Trainium2, batch-sharded over 8 cores.

x [32,6,512,512] f32, weight [16,6,5,5] (masked by the C3 connectivity
table), bias [16] -> out [32,16,508,508] f32.

Per-core scheme (4 images each), v3 — dx-paired 3-pass matmuls:
  - Host relayouts x per core to [H, C, BPC, W] and reads the device
    output as [OC, OH, BPC, OW]; every DMA is a <=3-dim access pattern
    (the HW limit) covering a full 6-row window across all 4 images.
  - Window = 6 output rows (needs 10 input rows). SBUF x slot holds two
    column-shifted copies of the window: copyA (partitions 0..59 =
    (row<10, ch<6) row-major) straight from DRAM, copyB (partitions
    64..123) = copyA shifted one column left, made by a DVE
    tensor_copy (partitions 60..63 are zeroed once; matmul base
    partitions must be 0/32/64-aligned so copyB sits at 64).
  - Each of 3 matmul passes contracts K=124 partitions at moving
    column offset 2t: copyA rows carry the dx=2t filter taps, copyB
    rows carry dx=2t+1 (zero for t=2). 3 passes replace the naive 5
    (one per kernel column): PE row-streams drop 5/8 -> 3/6 per output
    row, a 20% PE-time cut, and M=96 psum partitions = (oc,yy<6).
  - Per (window,img) one PSUM bank accumulates the 3 passes; ScalarE
    evicts 4 banks/window with fused bias into bf16 and issues the
    store; the output is upcast to f32 on host.
  - Per window: 1 load DMA + 1 DVE shift-copy + 12 matmuls + 1
    activation + 1 store DMA. Engine busy (sim): PE ~216us, DMA ~155us,
    ACT ~160us, DVE ~100us, HWDGE ~110us.
"""

import numpy as np

# LeNet-5 C3 connectivity: input maps feeding each of the 16 output maps.
MAP_S2 = [[0, 1, 2], [1, 2, 3], [2, 3, 4], [3, 4, 5], [0, 4, 5], [0, 1, 5],
          [0, 1, 2, 3], [1, 2, 3, 4], [2, 3, 4, 5], [0, 3, 4, 5], [0, 1, 4, 5],
          [0, 1, 2, 5], [0, 1, 3, 4], [1, 2, 4, 5], [0, 2, 3, 5],
          [0, 1, 2, 3, 4, 5]]

B, C, H, W = 32, 6, 512, 512
OC, KH, KW = 16, 5, 5
OH, OW = H - KH + 1, W - KW + 1  # 508, 508
NCORES = 8
BPC = B // NCORES   # 4 images per core
YB = 6              # output rows per window
RW = YB + KH - 1    # 10 input rows per window
M = OC * YB         # 96 psum partitions
KP = 124            # contraction partitions: A 0..59, zero 60..63, B 64..123
NPASS = 3           # dx pairs {0,1}, {2,3}, {4,-}
BW = BPC * W        # free elements per slot (img-major, w inner)

NW = 85             # windows per rep: y0 = 6w (w<84), tail y0=502
XW = 6              # x lanes
NSL = 4             # output slots
PSB = 8             # psum banks
LA = 4              # load lookahead (windows)

Y0S = [6 * w for w in range(NW - 1)] + [OH - YB]

_NC_CACHE = {}
# dt = moving/copy dtype for x, w_dt = stationary dtype. bfloat16 halves
# DMA traffic and doubles DVE copy rate; out_bf16 stores the output as
# bf16 (upcast on host). Both keep rel err ~2e-3 vs the 2e-2 gate.
CFG = {"dt": "bfloat16", "w_dt": None, "out_bf16": True}


def _np_dt(name):
    if name in (None, "float32", "float32r"):
        return np.float32
    import ml_dtypes
    return np.dtype(getattr(ml_dtypes, name))


def _conn_mask():
    m = np.zeros((OC, C), dtype=np.float32)
    for i, conn in enumerate(MAP_S2):
        m[i, conn] = 1.0
    return m


def build_nc(dt_name="bfloat16", w_dt_name=None, reps=1, out_bf16=True):
    import concourse.bass as bass
    import concourse.mybir as mybir
    from contextlib import ExitStack

    MMDT = getattr(mybir.dt, dt_name)
    WDT = getattr(mybir.dt, w_dt_name or dt_name)
    F32 = mybir.dt.float32
    ODT = mybir.dt.bfloat16 if out_bf16 else F32
    IDENT = mybir.ActivationFunctionType.Identity
    TW = reps * NW        # total windows (reps amplify work for timing)

    nc = bass.Bass()
    x_t = nc.dram_tensor("x", [H, C, BPC, W], MMDT, kind="ExternalInput")
    w_t = nc.dram_tensor("wstat", [KP, NPASS * M], WDT, kind="ExternalInput")
    b_t = nc.dram_tensor("biasrep", [M, 1], F32, kind="ExternalInput")
    z_t = nc.dram_tensor("zpad", [4, XW * BPC * W], MMDT,
                         kind="ExternalInput")
    out_t = nc.dram_tensor("out", [OC, OH, BPC, OW], ODT,
                           kind="ExternalOutput")

    with ExitStack() as ctx:
        wt = ctx.enter_context(nc.sbuf_tensor("wt", [KP, NPASS * M], WDT))
        bt = ctx.enter_context(nc.sbuf_tensor("bt", [M, 1], F32))
        xt = ctx.enter_context(nc.sbuf_tensor("xt", [128, XW, BW], MMDT))
        ot = ctx.enter_context(nc.sbuf_tensor("ot", [M, NSL, BPC, OW], ODT))
        pst = ctx.enter_context(nc.psum_tensor("pst", [M, PSB, 512], F32))
        wt_sem = ctx.enter_context(nc.semaphore("wt_sem"))
        bt_sem = ctx.enter_context(nc.semaphore("bt_sem"))
        # per-lane x sems: same-lane DMA completions are ordered through
        # the lane-recycle chain, so cumulative thresholds are race-free.
        x_sems = [ctx.enter_context(nc.semaphore(f"x_sem{i}"))
                  for i in range(XW)]
        st_sems = [ctx.enter_context(nc.semaphore(f"st_sem{i}"))
                   for i in range(NSL)]
        cp_sem = ctx.enter_context(nc.semaphore("cp_sem"))
        ps_sem = ctx.enter_context(nc.semaphore("ps_sem"))
        ev_sem = ctx.enter_context(nc.semaphore("ev_sem"))
        block = ctx.enter_context(nc.Block())

        XFREE = XW * BW  # xt free elements per partition

        def load_win(sync, wp):
            y0 = Y0S[wp % NW]
            lane = wp % XW
            # dst partition = r*6 + c (one partition-crossing dim); src
            # [H, C, BPC, W] slice iterates (r, c, (img,w)) to match.
            dst = bass.AP(xt, lane * BW, [[XFREE, C * RW], [1, BW]])
            sync.dma_start(
                out=dst, in_=x_t[y0:y0 + RW, :, :, :],
            ).then_inc(x_sems[lane], 16)

        @block.sync
        def _(sync):
            sync.dma_start(out=wt[:, :], in_=w_t[:, :]).then_inc(wt_sem, 16)
            # zero the 60..63 partition hole once: stationary rows there
            # are zero, but 0 * uninitialized-NaN would poison PSUM. A DMA
            # (not a DVE memset) keeps it off the copy critical path.
            sync.dma_start(out=xt[60:64, :, :], in_=z_t[:, :]
                           ).then_inc(wt_sem, 16)
            # copyB's last flat element per lane slot (BW-1) is outside
            # the shift-copy's write range but inside pass 2's read range
            # (x stationary rows there are zero): zero it once too.
            tail = bass.AP(xt, 64 * XFREE + BW - 1,
                           [[XFREE, C * RW], [BW, XW], [1, 1]])
            with nc.allow_non_contiguous_dma(reason="360 single-element"):
                sync.dma_start(out=tail, in_=z_t[0, 0:C * RW * XW]
                               ).then_inc(wt_sem, 16)
            sync.dma_start(out=bt[:, :], in_=b_t[:, :]).then_inc(bt_sem, 16)
            for wp in range(min(LA, TW)):
                load_win(sync, wp)
            # stores issue from SP, not ACT: the activation engine-wait
            # plus HWDGE store issue on one SEQ would exceed the PE
            # window time and become the critical path.
            for w in range(TW):
                wp = w + LA
                if wp < TW:
                    # lane (wp%XW) last read by window wp-XW's matmuls
                    if wp >= XW:
                        sync.wait_ge(ps_sem, BPC * (wp - XW + 1))
                    load_win(sync, wp)
                sync.wait_ge(ev_sem, w + 1)
                y0 = Y0S[w % NW]
                dst = bass.AP(
                    out_t, y0 * BPC * OW,
                    [[OH * BPC * OW, OC], [BPC * OW, YB], [1, BPC * OW]],
                )
                sync.dma_start(out=dst, in_=ot[:, w % NSL, :, :]
                               ).then_inc(st_sems[w % NSL], 16)

        @block.vector
        def _(vector):
            for w in range(TW):
                lane = w % XW
                vector.wait_ge(x_sems[lane], 16 * (w // XW + 1))
                # copyB = copyA shifted one column left; the last element
                # (flat BW-1) is never read: pass t reads cols 2t..2t+507
                # within each image's 512-block, 2t+507 <= 511.
                vector.tensor_copy(
                    xt[64:64 + C * RW, lane, 0:BW - 1],
                    xt[0:C * RW, lane, 1:BW],
                ).then_inc(cp_sem, 1)

        @block.tensor
        def _(tensor):
            tensor.wait_ge(wt_sem, 48)  # wstat + zpad slab + zpad tail
            for w in range(TW):
                lane = w % XW
                tensor.wait_ge(cp_sem, w + 1)
                # pass-major: each stationary loads once per 4 matmuls
                for t in range(NPASS):
                    for img in range(BPC):
                        if t == 0 and img == 0 and w >= 2:
                            # banks (w%2)*4.. last read by ACT window w-2
                            tensor.wait_ge(ev_sem, w - 1)
                        mm = tensor.matmul(
                            pst[:, (w % 2) * BPC + img, 0:OW],
                            wt[:, t * M:(t + 1) * M],
                            xt[0:KP, lane, img * W + 2 * t:
                               img * W + 2 * t + OW],
                            start=(t == 0),
                            stop=(t == NPASS - 1),
                        )
                        if t == NPASS - 1:
                            mm.then_inc(ps_sem, 1)

        @block.scalar
        def _(scalar):
            scalar.wait_ge(bt_sem, 16)
            for w in range(TW):
                scalar.wait_ge(ps_sem, BPC * (w + 1))
                sl = w % NSL
                if w >= NSL:
                    scalar.wait_ge(st_sems[sl], 16 * (w // NSL))
                b0 = (w % 2) * BPC
                scalar.activation(
                    ot[:, sl, :, :], pst[:, b0:b0 + BPC, 0:OW], IDENT,
                    bias=bt[:, :], scale=1.0,
                ).then_inc(ev_sem, 1)

    return nc


def _prep_wstat(weight):
    """[124, 3*96]: row r*6+c carries the dx=2t tap band, row 64+r*6+c
    the dx=2t+1 band (zero for t=2): wstat[r*6+c, t*96+oc*6+yy] =
    wm[oc, c, r-yy, 2t] for 0 <= r-yy < 5."""
    wm = (np.asarray(weight, dtype=np.float32) * _conn_mask()[:, :, None, None])
    wst = np.zeros((KP, NPASS * M), dtype=np.float32)
    for t in range(NPASS):
        for r in range(RW):
            for yy in range(YB):
                dy = r - yy
                if not 0 <= dy < KH:
                    continue
                for c in range(C):
                    col = t * M + np.arange(OC) * YB + yy
                    wst[r * C + c, col] = wm[:, c, dy, 2 * t]
                    if 2 * t + 1 < KW:
                        wst[64 + r * C + c, col] = wm[:, c, dy, 2 * t + 1]
    return np.ascontiguousarray(wst).astype(_np_dt(CFG["w_dt"] or CFG["dt"]))


def _make_in_maps(x, weight, bias):
    """Full inputs -> per-core in_maps (x relaid out to [H, C, BPC, W])."""
    xd = np.asarray(x, dtype=np.float32).astype(_np_dt(CFG["dt"]))
    wstat = _prep_wstat(weight)
    biasrep = np.ascontiguousarray(
        np.repeat(np.asarray(bias, dtype=np.float32), YB).reshape(M, 1))
    zpad = np.zeros((4, XW * BPC * W), dtype=_np_dt(CFG["dt"]))
    in_maps = []
    for c in range(NCORES):
        xc = np.ascontiguousarray(
            xd[c * BPC:(c + 1) * BPC].transpose(2, 1, 0, 3))
        in_maps.append({"x": xc, "wstat": wstat, "biasrep": biasrep,
                        "zpad": zpad})
    return in_maps


def kernel(x, weight, bias):
    from concourse.bass_utils import run_bass_kernel_spmd

    key = (CFG["dt"], CFG["w_dt"], CFG["out_bf16"])
    if _NC_CACHE.get("key") != key:
        _NC_CACHE["nc"] = build_nc(dt_name=CFG["dt"], w_dt_name=CFG["w_dt"],
                                   out_bf16=CFG["out_bf16"])
        _NC_CACHE["key"] = key
    nc = _NC_CACHE["nc"]

    in_maps = _make_in_maps(x, weight, bias)
    res = run_bass_kernel_spmd(nc, in_maps, list(range(NCORES)))
    # device out is [OC, OH, BPC, OW] (maybe bf16) -> [BPC, OC, OH, OW] f32
    return np.concatenate(
        [np.asarray(res.results[c]["out"]).astype(np.float32)
         .transpose(2, 0, 1, 3) for c in range(NCORES)],
        axis=0)


# revision 32
# speedup vs baseline: 12.2169x; 1.6737x over previous
"""LeNet-C3-style masked 5x5 VALID conv on Trainium2, batch-sharded over 8 cores.

x [32,6,512,512] f32, weight [16,6,5,5] (masked by the C3 connectivity
table), bias [16] -> out [32,16,508,508] f32.

Per-core scheme (4 images each), v3 — dx-paired 3-pass matmuls:
  - Host relayouts x per core to [H, C, BPC, W] and reads the device
    output as [OC, OH, BPC, OW]; every DMA is a <=3-dim access pattern
    (the HW limit) covering a full 6-row window across all 4 images.
  - Window = 6 output rows (needs 10 input rows). SBUF x slot holds two
    column-shifted copies of the window: copyA (partitions 0..59 =
    (row<10, ch<6) row-major) straight from DRAM, copyB (partitions
    64..123) = copyA shifted one column left, made by a DVE
    tensor_copy (partitions 60..63 are zeroed once; matmul base
    partitions must be 0/32/64-aligned so copyB sits at 64).
  - Each of 3 matmul passes contracts K=124 partitions at moving
    column offset 2t: copyA rows carry the dx=2t filter taps, copyB
    rows carry dx=2t+1 (zero for t=2). 3 passes replace the naive 5
    (one per kernel column): PE row-streams drop 5/8 -> 3/6 per output
    row, a 20% PE-time cut, and M=96 psum partitions = (oc,yy<6).
  - Per (window,img) one PSUM bank accumulates the 3 passes; ScalarE
    evicts 4 banks/window with fused bias into bf16 and issues the
    store; the output is upcast to f32 on host.
  - Per window: 1 load DMA + 1 DVE shift-copy + 12 matmuls + 1
    activation + 1 store DMA. Engine busy (sim): PE ~216us, DMA ~155us,
    ACT ~160us, DVE ~100us, HWDGE ~110us.
"""

import numpy as np

# LeNet-5 C3 connectivity: input maps feeding each of the 16 output maps.
MAP_S2 = [[0, 1, 2], [1, 2, 3], [2, 3, 4], [3, 4, 5], [0, 4, 5], [0, 1, 5],
          [0, 1, 2, 3], [1, 2, 3, 4], [2, 3, 4, 5], [0, 3, 4, 5], [0, 1, 4, 5],
          [0, 1, 2, 5], [0, 1, 3, 4], [1, 2, 4, 5], [0, 2, 3, 5],
          [0, 1, 2, 3, 4, 5]]

B, C, H, W = 32, 6, 512, 512
OC, KH, KW = 16, 5, 5
OH, OW = H - KH + 1, W - KW + 1  # 508, 508
NCORES = 8
BPC = B // NCORES   # 4 images per core
YB = 6              # output rows per window
RW = YB + KH - 1    # 10 input rows per window
M = OC * YB         # 96 psum partitions
KP = 124            # contraction partitions: A 0..59, zero 60..63, B 64..123
NPASS = 3           # dx pairs {0,1}, {2,3}, {4,-}
BW = BPC * W        # free elements per slot (img-major, w inner)

NW = 85             # windows per rep: y0 = 6w (w<84), tail y0=502
XW = 6              # x lanes
NSL = 4             # output slots
PSB = 8             # psum banks
LA = 4              # load lookahead (windows)

Y0S = [6 * w for w in range(NW - 1)] + [OH - YB]

_NC_CACHE = {}
# dt = moving/copy dtype for x, w_dt = stationary dtype. bfloat16 halves
# DMA traffic and doubles DVE copy rate; out_bf16 stores the output as
# bf16 (upcast on host). Both keep rel err ~2e-3 vs the 2e-2 gate.
CFG = {"dt": "bfloat16", "w_dt": None, "out_bf16": True}


def _np_dt(name):
    if name in (None, "float32", "float32r"):
        return np.float32
    import ml_dtypes
    return np.dtype(getattr(ml_dtypes, name))


def _conn_mask():
    m = np.zeros((OC, C), dtype=np.float32)
    for i, conn in enumerate(MAP_S2):
        m[i, conn] = 1.0
    return m


def build_nc(dt_name="bfloat16", w_dt_name=None, reps=1, out_bf16=True):
    import concourse.bass as bass
    import concourse.mybir as mybir
    from contextlib import ExitStack

    MMDT = getattr(mybir.dt, dt_name)
    WDT = getattr(mybir.dt, w_dt_name or dt_name)
    F32 = mybir.dt.float32
    ODT = mybir.dt.bfloat16 if out_bf16 else F32
    IDENT = mybir.ActivationFunctionType.Identity
    TW = reps * NW        # total windows (reps amplify work for timing)

    nc = bass.Bass()
    x_t = nc.dram_tensor("x", [H, C, BPC, W], MMDT, kind="ExternalInput")
    w_t = nc.dram_tensor("wstat", [KP, NPASS * M], WDT, kind="ExternalInput")
    b_t = nc.dram_tensor("biasrep", [M, 1], F32, kind="ExternalInput")
    z_t = nc.dram_tensor("zpad", [4, XW * BPC * W], MMDT,
                         kind="ExternalInput")
    out_t = nc.dram_tensor("out", [OC, OH, BPC, OW], ODT,
                           kind="ExternalOutput")

    with ExitStack() as ctx:
        wt = ctx.enter_context(nc.sbuf_tensor("wt", [KP, NPASS * M], WDT))
        bt = ctx.enter_context(nc.sbuf_tensor("bt", [M, 1], F32))
        xt = ctx.enter_context(nc.sbuf_tensor("xt", [128, XW, BW], MMDT))
        ot = ctx.enter_context(nc.sbuf_tensor("ot", [M, NSL, BPC, OW], ODT))
        pst = ctx.enter_context(nc.psum_tensor("pst", [M, PSB, 512], F32))
        wt_sem = ctx.enter_context(nc.semaphore("wt_sem"))
        bt_sem = ctx.enter_context(nc.semaphore("bt_sem"))
        # per-lane x sems: same-lane DMA completions are ordered through
        # the lane-recycle chain, so cumulative thresholds are race-free.
        x_sems = [ctx.enter_context(nc.semaphore(f"x_sem{i}"))
                  for i in range(XW)]
        st_sems = [ctx.enter_context(nc.semaphore(f"st_sem{i}"))
                   for i in range(NSL)]
        cp_sem = ctx.enter_context(nc.semaphore("cp_sem"))
        ps_sem = ctx.enter_context(nc.semaphore("ps_sem"))
        ev_sem = ctx.enter_context(nc.semaphore("ev_sem"))
        block = ctx.enter_context(nc.Block())

        XFREE = XW * BW  # xt free elements per partition

        def load_win(sync, wp):
            y0 = Y0S[wp % NW]
            lane = wp % XW
            # dst partition = r*6 + c (one partition-crossing dim); src
            # [H, C, BPC, W] slice iterates (r, c, (img,w)) to match.
            dst = bass.AP(xt, lane * BW, [[XFREE, C * RW], [1, BW]])
            sync.dma_start(
                out=dst, in_=x_t[y0:y0 + RW, :, :, :],
            ).then_inc(x_sems[lane], 16)

        @block.sync
        def _(sync):
            sync.dma_start(out=wt[:, :], in_=w_t[:, :]).then_inc(wt_sem, 16)
            # zero the 60..63 partition hole once: stationary rows there
            # are zero, but 0 * uninitialized-NaN would poison PSUM. A DMA
            # (not a DVE memset) keeps it off the copy critical path.
            sync.dma_start(out=xt[60:64, :, :], in_=z_t[:, :]
                           ).then_inc(wt_sem, 16)
            # copyB's last flat element per lane slot (BW-1) is outside
            # the shift-copy's write range but inside pass 2's read range
            # (x stationary rows there are zero): zero it once too.
            tail = bass.AP(xt, 64 * XFREE + BW - 1,
                           [[XFREE, C * RW], [BW, XW], [1, 1]])
            with nc.allow_non_contiguous_dma(reason="360 single-element"):
                sync.dma_start(out=tail, in_=z_t[0, 0:C * RW * XW]
                               ).then_inc(wt_sem, 16)
            sync.dma_start(out=bt[:, :], in_=b_t[:, :]).then_inc(bt_sem, 16)
            for wp in range(min(LA, TW)):
                load_win(sync, wp)
            # stores issue from SP, not ACT: the activation engine-wait
            # plus HWDGE store issue on one SEQ would exceed the PE
            # window time and become the critical path.
            for w in range(TW):
                wp = w + LA
                if wp < TW:
                    # lane (wp%XW) last read by window wp-XW's matmuls
                    if wp >= XW:
                        sync.wait_ge(ps_sem, BPC * (wp - XW + 1))
                    load_win(sync, wp)
                sync.wait_ge(ev_sem, w + 1)
                y0 = Y0S[w % NW]
                dst = bass.AP(
                    out_t, y0 * BPC * OW,
                    [[OH * BPC * OW, OC], [BPC * OW, YB], [1, BPC * OW]],
                )
                sync.dma_start(out=dst, in_=ot[:, w % NSL, :, :]
                               ).then_inc(st_sems[w % NSL], 16)

        @block.vector
        def _(vector):
            for w in range(TW):
                lane = w % XW
                vector.wait_ge(x_sems[lane], 16 * (w // XW + 1))
                # copyB = copyA shifted one column left; the last element
                # (flat BW-1) is never read: pass t reads cols 2t..2t+507
                # within each image's 512-block, 2t+507 <= 511.
                vector.tensor_copy(
                    xt[64:64 + C * RW, lane, 0:BW - 1],
                    xt[0:C * RW, lane, 1:BW],
                ).then_inc(cp_sem, 1)

        @block.tensor
        def _(tensor):
            tensor.wait_ge(wt_sem, 48)  # wstat + zpad slab + zpad tail
            for w in range(TW):
                lane = w % XW
                tensor.wait_ge(cp_sem, w + 1)
                # pass-major: each stationary loads once per 4 matmuls
                for t in range(NPASS):
                    for img in range(BPC):
                        if t == 0 and img == 0 and w >= 2:
                            # banks (w%2)*4.. last read by ACT window w-2
                            tensor.wait_ge(ev_sem, w - 1)
                        mm = tensor.matmul(
                            pst[:, (w % 2) * BPC + img, 0:OW],
                            wt[:, t * M:(t + 1) * M],
                            xt[0:KP, lane, img * W + 2 * t:
                               img * W + 2 * t + OW],
                            start=(t == 0),
                            stop=(t == NPASS - 1),
                        )
                        if t == NPASS - 1:
                            mm.then_inc(ps_sem, 1)

        @block.scalar
        def _(scalar):
            scalar.wait_ge(bt_sem, 16)
            for w in range(TW):
                scalar.wait_ge(ps_sem, BPC * (w + 1))
                sl = w % NSL
                if w >= NSL:
                    scalar.wait_ge(st_sems[sl], 16 * (w // NSL))
                b0 = (w % 2) * BPC
                scalar.activation(
                    ot[:, sl, :, :], pst[:, b0:b0 + BPC, 0:OW], IDENT,
                    bias=bt[:, :], scale=1.0,
                ).then_inc(ev_sem, 1)

    return nc


def _prep_wstat(weight):
    """[124, 3*96]: row r*6+c carries the dx=2t tap band, row 64+r*6+c
    the dx=2t+1 band (zero for t=2): wstat[r*6+c, t*96+oc*6+yy] =
    wm[oc, c, r-yy, 2t] for 0 <= r-yy < 5."""
    wm = (np.asarray(weight, dtype=np.float32) * _conn_mask()[:, :, None, None])
    wst = np.zeros((KP, NPASS * M), dtype=np.float32)
    for t in range(NPASS):
        for r in range(RW):
            for yy in range(YB):
                dy = r - yy
                if not 0 <= dy < KH:
                    continue
                for c in range(C):
                    col = t * M + np.arange(OC) * YB + yy
                    wst[r * C + c, col] = wm[:, c, dy, 2 * t]
                    if 2 * t + 1 < KW:
                        wst[64 + r * C + c, col] = wm[:, c, dy, 2 * t + 1]
    return np.ascontiguousarray(wst).astype(_np_dt(CFG["w_dt"] or CFG["dt"]))


def _make_in_maps(x, weight, bias):
    """Full inputs -> per-core in_maps (x relaid out to [H, C, BPC, W])."""
    xd = np.asarray(x, dtype=np.float32).astype(_np_dt(CFG["dt"]))
    wstat = _prep_wstat(weight)
    biasrep = np.ascontiguousarray(
        np.repeat(np.asarray(bias, dtype=np.float32), YB).reshape(M, 1))
    zpad = np.zeros((4, XW * BPC * W), dtype=_np_dt(CFG["dt"]))
    in_maps = []
    for c in range(NCORES):
        xc = np.ascontiguousarray(
            xd[c * BPC:(c + 1) * BPC].transpose(2, 1, 0, 3))
        in_maps.append({"x": xc, "wstat": wstat, "biasrep": biasrep,
                        "zpad": zpad})
    return in_maps


def kernel(x, weight, bias):
    from concourse.bass_utils import run_bass_kernel_spmd

    key = (CFG["dt"], CFG["w_dt"], CFG["out_bf16"])
    if _NC_CACHE.get("key") != key:
        _NC_CACHE["nc"] = build_nc(dt_name=CFG["dt"], w_dt_name=CFG["w_dt"],
                                   out_bf16=CFG["out_bf16"])
        _NC_CACHE["key"] = key
    nc = _NC_CACHE["nc"]

    in_maps = _make_in_maps(x, weight, bias)
    res = run_bass_kernel_spmd(nc, in_maps, list(range(NCORES)))
    # device out is [OC, OH, BPC, OW] (maybe bf16) -> [BPC, OC, OH, OW] f32
    return np.concatenate(
        [np.asarray(res.results[c]["out"]).astype(np.float32)
         .transpose(2, 0, 1, 3) for c in range(NCORES)],
        axis=0)


# revision 33
# speedup vs baseline: 12.5896x; 1.0305x over previous
"""LeNet-C3-style masked 5x5 VALID conv on Trainium2, batch-sharded over 8 cores.

x [32,6,512,512] f32, weight [16,6,5,5] (masked by the C3 connectivity
table), bias [16] -> out [32,16,508,508] f32.

Per-core scheme (4 images each), v3 — dx-paired 3-pass matmuls:
  - Host relayouts x per core to [H, C, BPC, W] and reads the device
    output as [OC, OH, BPC, OW]; every DMA is a <=3-dim access pattern
    (the HW limit) covering a full 6-row window across all 4 images.
  - Window = 6 output rows (needs 10 input rows). SBUF x slot holds two
    column-shifted copies of the window: copyA (partitions 0..59 =
    (row<10, ch<6) row-major) straight from DRAM, copyB (partitions
    64..123) = copyA shifted one column left, made by a DVE
    tensor_copy (partitions 60..63 are zeroed once; matmul base
    partitions must be 0/32/64-aligned so copyB sits at 64).
  - Each of 3 matmul passes contracts K=124 partitions at moving
    column offset 2t: copyA rows carry the dx=2t filter taps, copyB
    rows carry dx=2t+1 (zero for t=2). 3 passes replace the naive 5
    (one per kernel column): PE row-streams drop 5/8 -> 3/6 per output
    row, a 20% PE-time cut, and M=96 psum partitions = (oc,yy<6).
  - Per (window,img) one PSUM bank accumulates the 3 passes; ScalarE
    evicts 4 banks/window with fused bias into bf16 and issues the
    store; the output is upcast to f32 on host.
  - Per window: 1 load DMA + 1 DVE shift-copy + 12 matmuls + 1
    activation + 1 store DMA. Engine busy (sim): PE ~216us, DMA ~155us,
    ACT ~160us, DVE ~100us, HWDGE ~110us.
"""

import numpy as np

# LeNet-5 C3 connectivity: input maps feeding each of the 16 output maps.
MAP_S2 = [[0, 1, 2], [1, 2, 3], [2, 3, 4], [3, 4, 5], [0, 4, 5], [0, 1, 5],
          [0, 1, 2, 3], [1, 2, 3, 4], [2, 3, 4, 5], [0, 3, 4, 5], [0, 1, 4, 5],
          [0, 1, 2, 5], [0, 1, 3, 4], [1, 2, 4, 5], [0, 2, 3, 5],
          [0, 1, 2, 3, 4, 5]]

B, C, H, W = 32, 6, 512, 512
OC, KH, KW = 16, 5, 5
OH, OW = H - KH + 1, W - KW + 1  # 508, 508
NCORES = 8
BPC = B // NCORES   # 4 images per core
YB = 6              # output rows per window
RW = YB + KH - 1    # 10 input rows per window
M = OC * YB         # 96 psum partitions
KP = 124            # contraction partitions: A 0..59, zero 60..63, B 64..123
NPASS = 3           # dx pairs {0,1}, {2,3}, {4,-}
BW = BPC * W        # free elements per slot (img-major, w inner)

NW = 85             # windows per rep: y0 = 6w (w<84), tail y0=502
XW = 8              # x lanes
NSL = 6             # output slots
PSB = 8             # psum banks
LA = 4              # load lookahead (windows)

Y0S = [6 * w for w in range(NW - 1)] + [OH - YB]

_NC_CACHE = {}
# dt = moving/copy dtype for x, w_dt = stationary dtype. bfloat16 halves
# DMA traffic and doubles DVE copy rate; out_bf16 stores the output as
# bf16 (upcast on host). Both keep rel err ~2e-3 vs the 2e-2 gate.
CFG = {"dt": "bfloat16", "w_dt": None, "out_bf16": True}


def _np_dt(name):
    if name in (None, "float32", "float32r"):
        return np.float32
    import ml_dtypes
    return np.dtype(getattr(ml_dtypes, name))


def _conn_mask():
    m = np.zeros((OC, C), dtype=np.float32)
    for i, conn in enumerate(MAP_S2):
        m[i, conn] = 1.0
    return m


def build_nc(dt_name="bfloat16", w_dt_name=None, reps=1, out_bf16=True):
    import concourse.bass as bass
    import concourse.mybir as mybir
    from contextlib import ExitStack

    MMDT = getattr(mybir.dt, dt_name)
    WDT = getattr(mybir.dt, w_dt_name or dt_name)
    F32 = mybir.dt.float32
    ODT = mybir.dt.bfloat16 if out_bf16 else F32
    IDENT = mybir.ActivationFunctionType.Identity
    TW = reps * NW        # total windows (reps amplify work for timing)

    nc = bass.Bass()
    x_t = nc.dram_tensor("x", [H, C, BPC, W], MMDT, kind="ExternalInput")
    w_t = nc.dram_tensor("wstat", [KP, NPASS * M], WDT, kind="ExternalInput")
    b_t = nc.dram_tensor("biasrep", [M, 1], F32, kind="ExternalInput")
    z_t = nc.dram_tensor("zpad", [4, XW * BPC * W], MMDT,
                         kind="ExternalInput")
    out_t = nc.dram_tensor("out", [OC, OH, BPC, OW], ODT,
                           kind="ExternalOutput")

    with ExitStack() as ctx:
        wt = ctx.enter_context(nc.sbuf_tensor("wt", [KP, NPASS * M], WDT))
        bt = ctx.enter_context(nc.sbuf_tensor("bt", [M, 1], F32))
        xt = ctx.enter_context(nc.sbuf_tensor("xt", [128, XW, BW], MMDT))
        ot = ctx.enter_context(nc.sbuf_tensor("ot", [M, NSL, BPC, OW], ODT))
        pst = ctx.enter_context(nc.psum_tensor("pst", [M, PSB, 512], F32))
        wt_sem = ctx.enter_context(nc.semaphore("wt_sem"))
        bt_sem = ctx.enter_context(nc.semaphore("bt_sem"))
        # per-lane x sems: same-lane DMA completions are ordered through
        # the lane-recycle chain, so cumulative thresholds are race-free.
        x_sems = [ctx.enter_context(nc.semaphore(f"x_sem{i}"))
                  for i in range(XW)]
        st_sems = [ctx.enter_context(nc.semaphore(f"st_sem{i}"))
                   for i in range(NSL)]
        cp_sem = ctx.enter_context(nc.semaphore("cp_sem"))
        ps_sem = ctx.enter_context(nc.semaphore("ps_sem"))
        ev_sem = ctx.enter_context(nc.semaphore("ev_sem"))
        block = ctx.enter_context(nc.Block())

        XFREE = XW * BW  # xt free elements per partition

        def load_win(sync, wp):
            y0 = Y0S[wp % NW]
            lane = wp % XW
            # dst partition = r*6 + c (one partition-crossing dim); src
            # [H, C, BPC, W] slice iterates (r, c, (img,w)) to match.
            dst = bass.AP(xt, lane * BW, [[XFREE, C * RW], [1, BW]])
            sync.dma_start(
                out=dst, in_=x_t[y0:y0 + RW, :, :, :],
            ).then_inc(x_sems[lane], 16)

        @block.sync
        def _(sync):
            sync.dma_start(out=wt[:, :], in_=w_t[:, :]).then_inc(wt_sem, 16)
            # zero the 60..63 partition hole once: stationary rows there
            # are zero, but 0 * uninitialized-NaN would poison PSUM. A DMA
            # (not a DVE memset) keeps it off the copy critical path.
            sync.dma_start(out=xt[60:64, :, :], in_=z_t[:, :]
                           ).then_inc(wt_sem, 16)
            # copyB's last flat element per lane slot (BW-1) is outside
            # the shift-copy's write range but inside pass 2's read range
            # (x stationary rows there are zero): zero it once too.
            tail = bass.AP(xt, 64 * XFREE + BW - 1,
                           [[XFREE, C * RW], [BW, XW], [1, 1]])
            with nc.allow_non_contiguous_dma(reason="360 single-element"):
                sync.dma_start(out=tail, in_=z_t[0, 0:C * RW * XW]
                               ).then_inc(wt_sem, 16)
            sync.dma_start(out=bt[:, :], in_=b_t[:, :]).then_inc(bt_sem, 16)
            for wp in range(min(LA, TW)):
                load_win(sync, wp)
            # stores issue from SP, not ACT: the activation engine-wait
            # plus HWDGE store issue on one SEQ would exceed the PE
            # window time and become the critical path.
            for w in range(TW):
                wp = w + LA
                if wp < TW:
                    # lane (wp%XW) last read by window wp-XW's matmuls
                    if wp >= XW:
                        sync.wait_ge(ps_sem, BPC * (wp - XW + 1))
                    load_win(sync, wp)
                sync.wait_ge(ev_sem, w + 1)
                y0 = Y0S[w % NW]
                dst = bass.AP(
                    out_t, y0 * BPC * OW,
                    [[OH * BPC * OW, OC], [BPC * OW, YB], [1, BPC * OW]],
                )
                sync.dma_start(out=dst, in_=ot[:, w % NSL, :, :]
                               ).then_inc(st_sems[w % NSL], 16)

        @block.vector
        def _(vector):
            for w in range(TW):
                lane = w % XW
                vector.wait_ge(x_sems[lane], 16 * (w // XW + 1))
                # copyB = copyA shifted one column left; the last element
                # (flat BW-1) is never read: pass t reads cols 2t..2t+507
                # within each image's 512-block, 2t+507 <= 511.
                vector.tensor_copy(
                    xt[64:64 + C * RW, lane, 0:BW - 1],
                    xt[0:C * RW, lane, 1:BW],
                ).then_inc(cp_sem, 1)

        @block.tensor
        def _(tensor):
            tensor.wait_ge(wt_sem, 48)  # wstat + zpad slab + zpad tail
            for w in range(TW):
                lane = w % XW
                tensor.wait_ge(cp_sem, w + 1)
                # pass-major: each stationary loads once per 4 matmuls
                for t in range(NPASS):
                    for img in range(BPC):
                        if t == 0 and img == 0 and w >= 2:
                            # banks (w%2)*4.. last read by ACT window w-2
                            tensor.wait_ge(ev_sem, w - 1)
                        mm = tensor.matmul(
                            pst[:, (w % 2) * BPC + img, 0:OW],
                            wt[:, t * M:(t + 1) * M],
                            xt[0:KP, lane, img * W + 2 * t:
                               img * W + 2 * t + OW],
                            start=(t == 0),
                            stop=(t == NPASS - 1),
                        )
                        if t == NPASS - 1:
                            mm.then_inc(ps_sem, 1)

        @block.scalar
        def _(scalar):
            scalar.wait_ge(bt_sem, 16)
            for w in range(TW):
                scalar.wait_ge(ps_sem, BPC * (w + 1))
                sl = w % NSL
                if w >= NSL:
                    scalar.wait_ge(st_sems[sl], 16 * (w // NSL))
                b0 = (w % 2) * BPC
                scalar.activation(
                    ot[:, sl, :, :], pst[:, b0:b0 + BPC, 0:OW], IDENT,
                    bias=bt[:, :], scale=1.0,
                ).then_inc(ev_sem, 1)

    return nc


def _prep_wstat(weight):
    """[124, 3*96]: row r*6+c carries the dx=2t tap band, row 64+r*6+c
    the dx=2t+1 band (zero for t=2): wstat[r*6+c, t*96+oc*6+yy] =
    wm[oc, c, r-yy, 2t] for 0 <= r-yy < 5."""
    wm = (np.asarray(weight, dtype=np.float32) * _conn_mask()[:, :, None, None])
    wst = np.zeros((KP, NPASS * M), dtype=np.float32)
    for t in range(NPASS):
        for r in range(RW):
            for yy in range(YB):
                dy = r - yy
                if not 0 <= dy < KH:
                    continue
                for c in range(C):
                    col = t * M + np.arange(OC) * YB + yy
                    wst[r * C + c, col] = wm[:, c, dy, 2 * t]
                    if 2 * t + 1 < KW:
                        wst[64 + r * C + c, col] = wm[:, c, dy, 2 * t + 1]
    return np.ascontiguousarray(wst).astype(_np_dt(CFG["w_dt"] or CFG["dt"]))


def _make_in_maps(x, weight, bias):
    """Full inputs -> per-core in_maps (x relaid out to [H, C, BPC, W])."""
    xd = np.asarray(x, dtype=np.float32).astype(_np_dt(CFG["dt"]))
    wstat = _prep_wstat(weight)
    biasrep = np.ascontiguousarray(
        np.repeat(np.asarray(bias, dtype=np.float32), YB).reshape(M, 1))
    zpad = np.zeros((4, XW * BPC * W), dtype=_np_dt(CFG["dt"]))
    in_maps = []
    for c in range(NCORES):
        xc = np.ascontiguousarray(
            xd[c * BPC:(c + 1) * BPC].transpose(2, 1, 0, 3))
        in_maps.append({"x": xc, "wstat": wstat, "biasrep": biasrep,
                        "zpad": zpad})
    return in_maps


def kernel(x, weight, bias):
    from concourse.bass_utils import run_bass_kernel_spmd

    key = (CFG["dt"], CFG["w_dt"], CFG["out_bf16"])
    if _NC_CACHE.get("key") != key:
        _NC_CACHE["nc"] = build_nc(dt_name=CFG["dt"], w_dt_name=CFG["w_dt"],
                                   out_bf16=CFG["out_bf16"])
        _NC_CACHE["key"] = key
    nc = _NC_CACHE["nc"]

    in_maps = _make_in_maps(x, weight, bias)
    res = run_bass_kernel_spmd(nc, in_maps, list(range(NCORES)))
    # device out is [OC, OH, BPC, OW] (maybe bf16) -> [BPC, OC, OH, OW] f32
    return np.concatenate(
        [np.asarray(res.results[c]["out"]).astype(np.float32)
         .transpose(2, 0, 1, 3) for c in range(NCORES)],
        axis=0)
